# revision 8
# baseline (speedup 1.0000x reference)
"""TRN2 Bass kernel for nn_FFLayer: y = relu(l2norm_rows(x) @ W.T + b).

Strategy: data-parallel over the batch dim across 8 NeuronCores.
Each core gets a 1024-row shard of x (shipped pre-transposed as x^T so the
contraction dim lands on SBUF partitions), the full W (shipped as W^T), and b.

Per-core kernel (all matmuls in bf16: ~1.9e-3 rel-err vs 2e-2 tolerance):
  - norms: square x^T tiles on DVE (bf16 scratch), reduce over k with a
    ones-vector matmul on the PE into PSUM -> sqrt(+eps) on ACT.
  - GEMM: out[m, n-chunk] accumulated over 32 k-tiles; bias is folded in as a
    rank-1 matmul (norm+eps) x b so scale+bias+relu collapses into the single
    ACT drain pass: y = Relu(psum * s) with per-partition scale s = 1/(norm+eps).
"""
import sys

sys.path.insert(0, "/opt/trn_rl_repo")

import numpy as np

import concourse.bacc as bacc
import concourse.bass as bass
import concourse.mybir as mybir
import concourse.tile as tile
from concourse.bass_utils import run_bass_kernel_spmd

F32 = mybir.dt.float32
F32R = mybir.dt.bfloat16  # bf16 I/O: 1 cyc/row PE rate, half DMA, rel err ~1.9e-3 (tol 2e-2)
BF16 = mybir.dt.bfloat16
ACTF = mybir.ActivationFunctionType

N_CORES = 8
B, IN, OUT = 8192, 4096, 4096
MS = B // N_CORES          # 1024 rows per core
MT = MS // 128             # 8 m-tiles
KT = IN // 128             # 32 k-tiles
KQ = KT // 4               # k-tiles per W quarter-load
NCH = 512                  # n-chunk: 512 f32 = one full PSUM bank; halves matmul/DMA instr count
NC_N = OUT // NCH          # 16 chunks
EPS = 1e-4

_cached_nc = {}


def _build(reps=1, hw_loop=False):
    nc = bacc.Bacc("TRN2", target_bir_lowering=False, debug=False)

    # xs[mb*128 + p, kt*128 + m] = x_shard[mb*128 + m, kt*128 + p]
    # (per-m-block loads are fully contiguous)
    xs_d = nc.dram_tensor("xs", [MS, IN], F32R, kind="ExternalInput")
    # wt[c, q, p, j, n] = W[c*256 + n, (q*8 + j)*128 + p]
    # (per-quarter-chunk loads are fully contiguous)
    wt_d = nc.dram_tensor(
        "wt", [NC_N, 4, 128, KQ, NCH], F32R, kind="ExternalInput"
    )
    b_d = nc.dram_tensor("bias", [OUT], F32R, kind="ExternalInput")
    y_d = nc.dram_tensor("y", [MS, OUT], F32, kind="ExternalOutput")

    with tile.TileContext(nc) as tc:
        with (
            tc.tile_pool(name="xp", bufs=1) as xp,
            tc.tile_pool(name="wp", bufs=8) as wp,
            tc.tile_pool(name="sqp", bufs=1) as sqp,
            tc.tile_pool(name="op", bufs=2) as op,
            tc.tile_pool(name="rows", bufs=1) as rows,
            tc.tile_pool(name="npsum", bufs=2, space=bass.MemorySpace.PSUM) as npsum,
            tc.tile_pool(name="gpsum", bufs=6, space=bass.MemorySpace.PSUM) as gpsum,
        ):
            ones_f = rows.tile([128, 1], F32, tag="ones_f")
            ones_row = rows.tile([1, 128], BF16, tag="ones_row")
            nrow_f = rows.tile([1, MS], F32, tag="nrow_f")    # norm + eps (fp32)
            ncol = rows.tile([128, MT], F32, tag="ncol")      # norm + eps, [m%128, mt]
            s_col = rows.tile([128, MT], F32, tag="s_col")    # 1/(norm+eps)
            b_row = rows.tile([1, OUT], F32R, tag="b_row")    # bias (bf16)
            bb = rows.tile([128, OUT], F32, tag="bb")         # bias bcast across partitions

            nc.gpsimd.memset(ones_f[:], 1.0)
            nc.gpsimd.memset(ones_row[:], 1.0)

            # x^T m-blocks: one tile per m-block so matmul deps are exact
            xts = [None] * MT

            def load_x_block(mt):
                m0 = mt * 128
                t = xp.tile([128, KT, 128], F32R, tag=f"xt{mt}")
                nc.sync.dma_start(
                    t[:],
                    xs_d.ap()[m0 : m0 + 128, :].rearrange("p (kt m) -> p kt m", m=128),
                )
                xts[mt] = t

            def load_w_quarter(c, q):
                w = wp.tile([128, KQ, NCH], F32R, tag="wc")
                nc.sync.dma_start(w[:], wt_d.ap()[c, q])
                return w

            def build_bias_bcast():
                """bb[p, n] = b[n]: 8 rank-1 broadcast matmuls + psum->sbuf copies."""
                nc.sync.dma_start(
                    b_row[:], b_d.ap().rearrange("(o n) -> o n", o=1)
                )
                for i in range(OUT // NCH):
                    n0 = i * NCH
                    ps = gpsum.tile([128, NCH], F32, tag="gps")
                    nc.tensor.matmul(
                        ps[:], ones_row[:], b_row[0:1, n0 : n0 + NCH],
                        start=True, stop=True,
                    )
                    nc.scalar.activation(bb[:, n0 : n0 + NCH], ps[:], ACTF.Copy)

            def norms_for(mt):
                m0 = mt * 128
                sq = sqp.tile([128, KT, 128], BF16, tag="sq")
                nc.vector.tensor_mul(sq[:], xts[mt][:], xts[mt][:])
                red = sqp.tile([128, 128], F32, tag="red")
                nc.vector.tensor_reduce(
                    red[:],
                    sq[:].rearrange("p kt m -> p m kt"),
                    mybir.AxisListType.X,
                    mybir.AluOpType.add,
                )
                npt = npsum.tile([1, 128], F32, tag="np")
                nc.tensor.matmul(npt[:], ones_f[:], red[:], start=True, stop=True)
                # norm = sqrt(sumsq); then += eps in place
                nc.scalar.activation(nrow_f[0:1, m0 : m0 + 128], npt[:], ACTF.Sqrt)
                nc.scalar.activation(
                    nrow_f[0:1, m0 : m0 + 128],
                    nrow_f[0:1, m0 : m0 + 128],
                    ACTF.Copy,
                    bias=EPS,
                )
                # [1,128] free-run -> [128,1] partition fan-out, then reciprocal
                nc.gpsimd.dma_start(ncol[:, mt : mt + 1], nrow_f[0:1, m0 : m0 + 128])
                nc.vector.reciprocal(s_col[:, mt : mt + 1], ncol[:, mt : mt + 1])

            def gemm_group_pair(psA, psB, wq, mtA, mtB):
                """Two groups' k-matmuls interleaved: alternating PSUM banks
                hides any accumulate read-modify-write dependency bubble."""
                for kt in range(KT):
                    w_sl = wq[kt // KQ][:, kt % KQ, :]
                    nc.tensor.matmul(
                        psA[:], xts[mtA][:, kt, :], w_sl,
                        start=(kt == 0), stop=(kt == KT - 1),
                    )
                    nc.tensor.matmul(
                        psB[:], xts[mtB][:, kt, :], w_sl,
                        start=(kt == 0), stop=(kt == KT - 1),
                    )

            def drain(ps, mt, ncol0):
                m0 = mt * 128
                t = op.tile([128, NCH], F32, tag="t")
                nc.vector.scalar_tensor_tensor(
                    t[:], ps[:], s_col[:, mt : mt + 1],
                    bb[:, ncol0 : ncol0 + NCH],
                    mybir.AluOpType.mult, mybir.AluOpType.add,
                )
                o = op.tile([128, NCH], F32, tag="o")
                nc.scalar.activation(o[:], t[:], ACTF.Relu)
                nc.sync.dma_start(
                    y_d.ap()[m0 : m0 + 128, ncol0 : ncol0 + NCH], o[:]
                )

            def one_pass():
                # ---- startup: interleave x m-block loads with W chunk 0 load
                build_bias_bcast()
                load_x_block(0)
                w_cur = [load_w_quarter(0, q) for q in range(2)]
                load_x_block(1)
                w_cur += [load_w_quarter(0, q) for q in range(2, 4)]
                for mt in range(2, MT):
                    load_x_block(mt)

                # ---- 16 chunk passes; prefetch next chunk's W right after the
                # first group of the current pass (its slots freed a pass ago)
                for c in range(NC_N):
                    w_nxt = None
                    for mt0 in range(0, MT, 2):
                        if c == 0:
                            norms_for(mt0)
                            norms_for(mt0 + 1)
                        psA = gpsum.tile([128, NCH], F32, tag="gps")
                        psB = gpsum.tile([128, NCH], F32, tag="gps")
                        gemm_group_pair(psA, psB, w_cur, mt0, mt0 + 1)
                        drain(psA, mt0, c * NCH)
                        drain(psB, mt0 + 1, c * NCH)
                        if mt0 == 0 and c + 1 < NC_N:
                            w_nxt = [load_w_quarter(c + 1, q) for q in range(4)]
                    w_cur = w_nxt

            if hw_loop and reps > 1:
                # hardware loop: NEFF holds one pass + a back-edge barrier,
                # so the instruction stream stays small at any rep count
                with tc.For_i(0, reps, 1):
                    one_pass()
            else:
                for _rep in range(reps):
                    one_pass()

    nc.compile()
    return nc


def _get_nc(reps=1, hw_loop=False):
    key = (reps, hw_loop)
    if key not in _cached_nc:
        _cached_nc[key] = _build(reps, hw_loop)
    return _cached_nc[key]


def prep_inputs(x, W, b):
    import ml_dtypes

    bf16 = ml_dtypes.bfloat16
    x = np.asarray(x, dtype=np.float32).astype(bf16)
    W = np.asarray(W, dtype=np.float32).astype(bf16)
    b = np.asarray(b, dtype=np.float32).astype(bf16)
    # x: [core, mb, m, kt, p] -> [core, mb, p, kt, m]
    xs_all = np.ascontiguousarray(
        x.reshape(N_CORES, MT, 128, KT, 128).transpose(0, 1, 4, 3, 2)
    ).reshape(N_CORES, MS, IN)
    # W: [c, n, q, j, p] -> [c, q, p, j, n]
    w4 = np.ascontiguousarray(
        W.reshape(NC_N, NCH, 4, KQ, 128).transpose(0, 2, 4, 3, 1)
    )
    return [{"xs": xs_all[i], "wt": w4, "bias": b} for i in range(N_CORES)]


def kernel(x: np.ndarray, W: np.ndarray, b: np.ndarray, **run_kwargs) -> np.ndarray:
    nc = _get_nc()
    in_maps = prep_inputs(x, W, b)

    res = run_bass_kernel_spmd(nc, in_maps, list(range(N_CORES)), **run_kwargs)
    out = np.concatenate([res.results[i]["y"] for i in range(N_CORES)], axis=0)
    if run_kwargs:
        kernel.last_result = res
    return out



# revision 10
# speedup vs baseline: 1.0892x; 1.0892x over previous
"""TRN2 Bass kernel for nn_FFLayer: y = relu(l2norm_rows(x) @ W.T + b).

Strategy: data-parallel over the batch dim across 8 NeuronCores.
Each core gets a 1024-row shard of x (shipped pre-transposed as x^T so the
contraction dim lands on SBUF partitions), the full W (shipped as W^T), and b.

Per-core kernel (all matmuls in bf16: ~1.9e-3 rel-err vs 2e-2 tolerance):
  - norms: square x^T tiles on DVE (bf16 scratch), reduce over k with a
    ones-vector matmul on the PE into PSUM -> sqrt(+eps) on ACT.
  - GEMM: out[m, n-chunk] accumulated over 32 k-tiles; bias is folded in as a
    rank-1 matmul (norm+eps) x b so scale+bias+relu collapses into the single
    ACT drain pass: y = Relu(psum * s) with per-partition scale s = 1/(norm+eps).
"""
import sys

sys.path.insert(0, "/opt/trn_rl_repo")

import numpy as np

import concourse.bacc as bacc
import concourse.bass as bass
import concourse.mybir as mybir
import concourse.tile as tile
from concourse.bass_utils import run_bass_kernel_spmd

F32 = mybir.dt.float32
F32R = mybir.dt.bfloat16  # bf16 I/O: 1 cyc/row PE rate, half DMA, rel err ~1.9e-3 (tol 2e-2)
BF16 = mybir.dt.bfloat16
ACTF = mybir.ActivationFunctionType

N_CORES = 8
B, IN, OUT = 8192, 4096, 4096
MS = B // N_CORES          # 1024 rows per core
MT = MS // 128             # 8 m-tiles
KT = IN // 128             # 32 k-tiles
KQ = KT // 4               # k-tiles per W quarter-load
NCH = 512                  # n-chunk: 512 f32 = one full PSUM bank; halves matmul/DMA instr count
NC_N = OUT // NCH          # 16 chunks
EPS = 1e-4

_cached_nc = {}


def _build(reps=1, hw_loop=False):
    nc = bacc.Bacc("TRN2", target_bir_lowering=False, debug=False)

    # xs[mb*128 + p, kt*128 + m] = x_shard[mb*128 + m, kt*128 + p]
    # (per-m-block loads are fully contiguous)
    xs_d = nc.dram_tensor("xs", [MS, IN], F32R, kind="ExternalInput")
    # wt[c, q, p, j, n] = W[c*256 + n, (q*8 + j)*128 + p]
    # (per-quarter-chunk loads are fully contiguous)
    wt_d = nc.dram_tensor(
        "wt", [NC_N, 4, 128, KQ, NCH], F32R, kind="ExternalInput"
    )
    b_d = nc.dram_tensor("bias", [OUT], F32R, kind="ExternalInput")
    y_d = nc.dram_tensor("y", [MS, OUT], F32, kind="ExternalOutput")

    with tile.TileContext(nc) as tc:
        with (
            tc.tile_pool(name="xp", bufs=1) as xp,
            tc.tile_pool(name="wp", bufs=8) as wp,
            tc.tile_pool(name="sqp", bufs=1) as sqp,
            tc.tile_pool(name="op", bufs=2) as op,
            tc.tile_pool(name="rows", bufs=1) as rows,
            tc.tile_pool(name="npsum", bufs=2, space=bass.MemorySpace.PSUM) as npsum,
            tc.tile_pool(name="gpsum", bufs=6, space=bass.MemorySpace.PSUM) as gpsum,
        ):
            ones_f = rows.tile([128, 1], F32, tag="ones_f")
            ones_row = rows.tile([1, 128], BF16, tag="ones_row")
            nrow_f = rows.tile([1, MS], F32, tag="nrow_f")    # norm + eps (fp32)
            ncol = rows.tile([128, MT], F32, tag="ncol")      # norm + eps, [m%128, mt]
            s_col = rows.tile([128, MT], F32, tag="s_col")    # 1/(norm+eps)
            b_row = rows.tile([1, OUT], F32R, tag="b_row")    # bias (bf16)
            bb = rows.tile([128, OUT], F32, tag="bb")         # bias bcast across partitions

            nc.gpsimd.memset(ones_f[:], 1.0)
            nc.gpsimd.memset(ones_row[:], 1.0)

            # x^T m-blocks: one tile per m-block so matmul deps are exact
            xts = [None] * MT

            def load_x_block(mt):
                m0 = mt * 128
                t = xp.tile([128, KT, 128], F32R, tag=f"xt{mt}")
                nc.sync.dma_start(
                    t[:],
                    xs_d.ap()[m0 : m0 + 128, :].rearrange("p (kt m) -> p kt m", m=128),
                )
                xts[mt] = t

            def load_w_quarter(c, q):
                w = wp.tile([128, KQ, NCH], F32R, tag="wc")
                nc.sync.dma_start(w[:], wt_d.ap()[c, q])
                return w

            def build_bias_bcast():
                """bb[p, n] = b[n]: 8 rank-1 broadcast matmuls + psum->sbuf copies."""
                nc.sync.dma_start(
                    b_row[:], b_d.ap().rearrange("(o n) -> o n", o=1)
                )
                for i in range(OUT // NCH):
                    n0 = i * NCH
                    ps = gpsum.tile([128, NCH], F32, tag="gps")
                    nc.tensor.matmul(
                        ps[:], ones_row[:], b_row[0:1, n0 : n0 + NCH],
                        start=True, stop=True,
                    )
                    nc.scalar.activation(bb[:, n0 : n0 + NCH], ps[:], ACTF.Copy)

            def norms_for(mt):
                m0 = mt * 128
                sq = sqp.tile([128, KT, 128], BF16, tag="sq")
                nc.vector.tensor_mul(sq[:], xts[mt][:], xts[mt][:])
                red = sqp.tile([128, 128], F32, tag="red")
                nc.vector.tensor_reduce(
                    red[:],
                    sq[:].rearrange("p kt m -> p m kt"),
                    mybir.AxisListType.X,
                    mybir.AluOpType.add,
                )
                npt = npsum.tile([1, 128], F32, tag="np")
                nc.tensor.matmul(npt[:], ones_f[:], red[:], start=True, stop=True)
                # norm = sqrt(sumsq); then += eps in place
                nc.scalar.activation(nrow_f[0:1, m0 : m0 + 128], npt[:], ACTF.Sqrt)
                nc.scalar.activation(
                    nrow_f[0:1, m0 : m0 + 128],
                    nrow_f[0:1, m0 : m0 + 128],
                    ACTF.Copy,
                    bias=EPS,
                )
                # [1,128] free-run -> [128,1] partition fan-out, then reciprocal
                nc.gpsimd.dma_start(ncol[:, mt : mt + 1], nrow_f[0:1, m0 : m0 + 128])
                nc.vector.reciprocal(s_col[:, mt : mt + 1], ncol[:, mt : mt + 1])

            def gemm_group_multi(pss, wq, mts):
                """Interleave several groups' k-matmuls, rotating PSUM banks
                so each bank's accumulate RMW has slack to retire."""
                for kt in range(KT):
                    w_sl = wq[kt // KQ][:, kt % KQ, :]
                    for ps, mt in zip(pss, mts):
                        nc.tensor.matmul(
                            ps[:], xts[mt][:, kt, :], w_sl,
                            start=(kt == 0), stop=(kt == KT - 1),
                        )

            def drain(ps, mt, ncol0):
                m0 = mt * 128
                t = op.tile([128, NCH], F32, tag="t")
                nc.vector.scalar_tensor_tensor(
                    t[:], ps[:], s_col[:, mt : mt + 1],
                    bb[:, ncol0 : ncol0 + NCH],
                    mybir.AluOpType.mult, mybir.AluOpType.add,
                )
                o = op.tile([128, NCH], F32, tag="o")
                nc.scalar.activation(o[:], t[:], ACTF.Relu)
                nc.sync.dma_start(
                    y_d.ap()[m0 : m0 + 128, ncol0 : ncol0 + NCH], o[:]
                )

            def one_pass():
                # ---- startup: interleave x m-block loads with W chunk 0 load
                build_bias_bcast()
                load_x_block(0)
                w_cur = [load_w_quarter(0, q) for q in range(2)]
                load_x_block(1)
                w_cur += [load_w_quarter(0, q) for q in range(2, 4)]
                for mt in range(2, MT):
                    load_x_block(mt)

                # ---- 16 chunk passes; prefetch next chunk's W right after the
                # first group of the current pass (its slots freed a pass ago)
                for c in range(NC_N):
                    w_nxt = None
                    for gi, mts in enumerate([(0, 1, 2), (3, 4, 5), (6, 7)]):
                        if c == 0:
                            for mt in mts:
                                norms_for(mt)
                        pss = []
                        for _ in mts:
                            ps_t = gpsum.tile([128, NCH], F32, tag="gps")
                            pss.append(ps_t)
                        gemm_group_multi(pss, w_cur, mts)
                        for ps, mt in zip(pss, mts):
                            drain(ps, mt, c * NCH)
                        if gi == 0 and c + 1 < NC_N:
                            w_nxt = [load_w_quarter(c + 1, q) for q in range(4)]
                    w_cur = w_nxt

            if hw_loop and reps > 1:
                # hardware loop: NEFF holds one pass + a back-edge barrier,
                # so the instruction stream stays small at any rep count
                with tc.For_i(0, reps, 1):
                    one_pass()
            else:
                for _rep in range(reps):
                    one_pass()

    nc.compile()
    return nc


def _get_nc(reps=1, hw_loop=False):
    key = (reps, hw_loop)
    if key not in _cached_nc:
        _cached_nc[key] = _build(reps, hw_loop)
    return _cached_nc[key]


def prep_inputs(x, W, b):
    import ml_dtypes

    bf16 = ml_dtypes.bfloat16
    x = np.asarray(x, dtype=np.float32).astype(bf16)
    W = np.asarray(W, dtype=np.float32).astype(bf16)
    b = np.asarray(b, dtype=np.float32).astype(bf16)
    # x: [core, mb, m, kt, p] -> [core, mb, p, kt, m]
    xs_all = np.ascontiguousarray(
        x.reshape(N_CORES, MT, 128, KT, 128).transpose(0, 1, 4, 3, 2)
    ).reshape(N_CORES, MS, IN)
    # W: [c, n, q, j, p] -> [c, q, p, j, n]
    w4 = np.ascontiguousarray(
        W.reshape(NC_N, NCH, 4, KQ, 128).transpose(0, 2, 4, 3, 1)
    )
    return [{"xs": xs_all[i], "wt": w4, "bias": b} for i in range(N_CORES)]


def kernel(x: np.ndarray, W: np.ndarray, b: np.ndarray, **run_kwargs) -> np.ndarray:
    nc = _get_nc()
    in_maps = prep_inputs(x, W, b)

    res = run_bass_kernel_spmd(nc, in_maps, list(range(N_CORES)), **run_kwargs)
    out = np.concatenate([res.results[i]["y"] for i in range(N_CORES)], axis=0)
    if run_kwargs:
        kernel.last_result = res
    return out

